# revision 17
# baseline (speedup 1.0000x reference)
"""Bass/Tile kernel for a single causal attention head on 8 trn2 NeuronCores.

Problem: input [8, 2048, 768], Wq/Wk/Wv [768, 64] ->
  O = softmax(causal(Q K^T)/sqrt(64)) V, per batch.  [8, 2048, 64]

Sharding: data-parallel over batch; core b handles batch b. Weights replicated.

v2 dataflow (all-bf16 matmul path, range-pipelined):
  - Host casts x to bf16 and pre-concatenates weights [Wq|Wk], [Wv|Wq] in
    bf16: two weight DMAs, no on-device weight shuffling.
  - x^T arrives via 24 xbar DMA transposes ([512,128] -> [128,512] chunks,
    one per (query-range, d-block)), issued upfront; projections for range
    g start as soon as its 6 chunks land.
  - Projections accumulate [Wq|Wk]^T xT and [Wv|Qq]^T xT d-major into one
    2-bank PSUM tile per range: rows 0-63 Q^T/V^T, 64-127 K^T/Q^T(dup).
  - Attention per query range r (width 512): S^T pair matmuls (full
    k-tiles) -> one exp (ACT, 1/8 scale fused) per pair; diagonal k-tiles
    packed 3-into-[128,1024] + one [128,256]; invalid (q < k) halves of
    diagonal blocks zeroed by GPSIMD affine_select post-exp.
  - proj(r+1) runs on PE between S(r) and O(r), hiding the exp latency;
    V-transposes for r+1 run after O(r), hiding their DVE-copy dependency.
  - O^T accumulates V_aug_j^T @ expS^T in PSUM; V_aug's ones column makes
    row 64 the softmax denominator.  O^T is PE-transposed back, scaled by
    1/rowsum, and stored with one batched DMA per range.

PSUM budget (8 banks): mm (proj accum / S pairs / S diag) 3x2 + s1
(diag-256 / V-transp / out-transp) 1 + po (O^T accum) 1.
"""

import numpy as np

import concourse.tile as tile
from concourse import bacc, mybir
from concourse.bass_utils import run_bass_kernel_spmd
from concourse.masks import make_identity

P = 128
N = 2048
D = 768
H = 64
NT = N // P   # 16 n-tiles
DT = D // P   # 6 d-tiles
W = 512       # q-range width
QR = N // W   # 4 q-ranges
F32 = mybir.dt.float32
BF16 = mybir.dt.bfloat16
MMDT = BF16
BF16_X = True

# Three of the four diagonal tiles (widths 512, 384, 128) pack into one
# [128, 1024] psum; the 256-wide one (jj=2) gets its own narrow tile.
DIAG_PACK = {0: (0, 512), 1: (512, 384), 3: (896, 128)}  # jj -> (off, width)
DIAG_TOT = 1024


def build_kernel(reps=1):
    nc = bacc.Bacc(name="attn_head")
    x_d = nc.dram_tensor("x", [P, DT, N], BF16, kind="ExternalInput")
    w2_d = nc.dram_tensor("W2", [P, 2, DT, 2 * H], BF16,
                          kind="ExternalInput")
    out_d = nc.dram_tensor("out", [N, H], F32, kind="ExternalOutput")

    Exp = mybir.ActivationFunctionType.Exp

    with tile.TileContext(nc) as tc:
        with (
            tc.tile_pool(name="persist", bufs=1) as persist,
            tc.tile_pool(name="xpool", bufs=1) as xpool,
            tc.tile_pool(name="work", bufs=3) as work,
            tc.tile_pool(name="psum", bufs=1, space="PSUM") as psum,
        ):
            # warm the ACT exp table while DMAs run
            dummy = persist.tile([P, 1], F32)
            nc.vector.memset(dummy[:], 0.0)
            nc.scalar.activation(dummy[:], dummy[:], Exp)

            ident = persist.tile([P, P], F32)
            make_identity(nc, ident[:])
            ident_r = persist.tile([P, P], MMDT)
            nc.vector.tensor_copy(out=ident_r[:], in_=ident[:])

            ones_col = persist.tile([P, 1], F32)
            nc.vector.memset(ones_col[:], 1.0)

            xT = xpool.tile([P, DT, N], BF16)      # x^T: [d%128, d//128, n]
            QK_sb = persist.tile([P, N], MMDT)       # rows 0-63 Q^T, 64- K^T
            VQ_sb = persist.tile([P, N], MMDT)       # rows 0-63 V^T, 64- Q^T
            Vb = persist.tile([P, NT, H + 1], MMDT)  # V tiles + ones col
            nc.vector.tensor_copy(
                out=Vb[:, :, H],
                in_=ones_col[:, 0].to_broadcast((P, NT)),
            )

            # weights as [128, 2, 6, 128]: partition = d%128, tile = d//128,
            # pre-concatenated on host ([Wq|Wk], [Wv|Wq]) so one matmul
            # emits two 64-row outputs; one DMA loads both.
            w_all = persist.tile([P, 2, DT, 2 * H], BF16)
            w_qk = w_all[:, 0]
            w_vq = w_all[:, 1]

            for rep in range(reps):
                # x arrives host-transposed in the xT layout [p, t, n]:
                # plain full-bandwidth DMAs, chunked per query range so
                # projections start as soon as their chunk lands.  Weight
                # halves bracket the first chunk so the QK stream can start
                # right after xt0 and the VQ stream right after.
                def xt_dma(g):
                    nc.sync.dma_start(
                        out=xT[:, :, g * W:(g + 1) * W],
                        in_=x_d[:, :, g * W:(g + 1) * W],
                    )
                if rep == 0:
                    nc.sync.dma_start(out=w_all[:, 0], in_=w2_d[:, 0])
                xt_dma(0)
                if rep == 0:
                    nc.sync.dma_start(out=w_all[:, 1], in_=w2_d[:, 1])
                for g in range(1, QR):
                    xt_dma(g)

                KTd = QK_sb[H:P, :]   # K^T on partitions 64-127
                QTd = VQ_sb[H:P, :]   # Q^T duplicate on partitions 64-127

                def proj_mm(r):
                    ns = slice(r * W, (r + 1) * W)
                    prj = psum.tile([P, 2, W], F32, tag="mm", bufs=3,
                                    name=f"prj_{rep}_{r}")
                    for si, w_t in ((0, w_qk), (1, w_vq)):
                        for d_i in range(DT):
                            kw = dict(start=(d_i == 0), stop=(d_i == DT - 1))
                            nc.tensor.matmul(prj[:, si], w_t[:, d_i],
                                             xT[:, d_i, ns], **kw)
                    nc.vector.tensor_copy(out=QK_sb[:, ns], in_=prj[:, 0])
                    nc.vector.tensor_copy(out=VQ_sb[:, ns], in_=prj[:, 1])

                def vtrans(r):
                    # V^T tiles -> V_aug [128, 65] per k-tile (batched copy)
                    pvt = psum.tile([P, 4, H], MMDT, tag="s1", bufs=1,
                                    name=f"pvt_{rep}_{r}")
                    for i in range(4):
                        nc.tensor.transpose(
                            pvt[:, i, :],
                            VQ_sb[0:H, r * W + i * P:r * W + (i + 1) * P],
                            ident_r[:H, :H],
                        )
                    nc.vector.tensor_copy(out=Vb[:, 4 * r:4 * r + 4, 0:H],
                                          in_=pvt[:])

                proj_mm(0)
                proj_mm(1)
                vtrans(0)

                for r in range(QR):
                    qs = slice(r * W, (r + 1) * W)
                    last_r = (r == QR - 1)
                    pair_es = []
                    diag = {}

                    def s_pairs(r=r, qs=qs, pair_es=pair_es):
                        for jp in range(2 * r):
                            ps2 = psum.tile([P, 2, W], F32, tag="mm", bufs=3,
                                            name=f"ps2_{rep}_{r}_{jp}")
                            es2 = work.tile([P, 2, W], MMDT, tag="es2",
                                            bufs=6, name=f"es2_{rep}_{r}_{jp}")
                            for u in range(2):
                                j = 2 * jp + u
                                nc.tensor.matmul(
                                    ps2[:, u, :], KTd[:, j * P:(j + 1) * P],
                                    QTd[:, qs], start=True, stop=True,
                                )
                            nc.scalar.activation(es2[:], ps2[:], Exp,
                                                 scale=0.125)
                            pair_es.append(es2)

                    def s_diag(r=r, diag=diag):
                        # diagonal k-tiles: 3 packed in [128, 1024] + one
                        # [128, 256]; invalid (q < k) halves of the diagonal
                        # 128x128 blocks zeroed post-exp on GPSIMD.
                        psd = psum.tile([P, DIAG_TOT], F32, tag="mm", bufs=3,
                                        name=f"psd_{rep}_{r}")
                        esd = work.tile([P, DIAG_TOT], MMDT, tag="esd",
                                        bufs=4, name=f"esd_{rep}_{r}")
                        ps1 = psum.tile([P, 256], F32, tag="s1", bufs=1,
                                        name=f"ps1_{rep}_{r}")
                        es1 = work.tile([P, 256], MMDT, tag="es1", bufs=4,
                                        name=f"es1_{rep}_{r}")
                        for jj in range(4):
                            j = 4 * r + jj
                            if jj == 2:
                                sv = ps1[:, :]
                            else:
                                poff, wd = DIAG_PACK[jj]
                                sv = psd[:, poff:poff + wd]
                            nc.tensor.matmul(
                                sv,
                                KTd[:, j * P:(j + 1) * P],
                                QTd[:, r * W + jj * P:(r + 1) * W],
                                start=True, stop=True,
                            )
                        nc.scalar.activation(esd[:, 0:W], psd[:, 0:W], Exp,
                                             scale=0.125)
                        nc.scalar.activation(esd[:, W:], psd[:, W:], Exp,
                                             scale=0.125)
                        nc.scalar.activation(es1[:], ps1[:], Exp, scale=0.125)
                        for jj in range(4):
                            ev = es1[:, 0:P] if jj == 2 else (
                                esd[:, DIAG_PACK[jj][0]:
                                     DIAG_PACK[jj][0] + P])
                            nc.gpsimd.affine_select(
                                out=ev, in_=ev,
                                compare_op=mybir.AluOpType.is_ge,
                                fill=0.0, base=0,
                                pattern=[[1, P]], channel_multiplier=-1,
                            )
                        diag["esd"] = esd
                        diag["es1"] = es1

                    def o_pairs(po, first, last, r=r, pair_es=pair_es):
                        n = 2 * r
                        for jp in range(n):
                            for u in range(2):
                                j = 2 * jp + u
                                nc.tensor.matmul(
                                    po[:], Vb[:, j, :], pair_es[jp][:, u, :],
                                    start=(first and j == 0),
                                    stop=(last and jp == n - 1 and u == 1),
                                    skip_group_check=True,
                                )

                    def o_diag(po, first, last, r=r, diag=diag):
                        for jj in range(4):
                            j = 4 * r + jj
                            if jj == 2:
                                rhs = diag["es1"][:, :]
                            else:
                                poff, wd = DIAG_PACK[jj]
                                rhs = diag["esd"][:, poff:poff + wd]
                            nc.tensor.matmul(
                                po[:, jj * P:],
                                Vb[:, j, :],
                                rhs,
                                start=(first and jj == 0),
                                stop=(last and jj == 3),
                                skip_group_check=True,
                            )

                    # ---- S matmuls + exp; diag first on the last range so
                    # the exp->select->O chain is off the tail. proj(r+2)
                    # fills the exp latency on PE.
                    if last_r:
                        s_diag()
                        s_pairs()
                    else:
                        s_pairs()
                        s_diag()
                    if r + 2 < QR:
                        proj_mm(r + 2)

                    # ---- O^T accumulation ------------------------------
                    po = psum.tile([H + 1, W], F32, tag="po", bufs=1,
                                   name=f"po_{rep}_{r}")
                    if last_r:
                        o_diag(po, first=True, last=False)
                        o_pairs(po, first=False, last=True)
                    else:
                        o_pairs(po, first=(r > 0), last=False)
                        o_diag(po, first=(r == 0), last=True)

                    # ---- V-transposes for r+1 (dep ready by now) -------
                    if r + 1 < QR:
                        vtrans(r + 1)

                    # ---- normalize + batched output --------------------
                    # O^T transposes go into two 2-bank psum tiles so the
                    # four groups land in four distinct banks and don't
                    # serialize behind each other's readers.
                    ot = work.tile([H + 1, W], F32, tag="ot", bufs=2,
                                   name=f"ot_{rep}_{r}")
                    nc.gpsimd.tensor_copy(out=ot[:], in_=po[:])
                    pf_a = psum.tile([P, 2, W], F32, tag="mm", bufs=3,
                                     name=f"pfa_{rep}_{r}")
                    pf_b = psum.tile([P, 2, W], F32, tag="mm", bufs=3,
                                     name=f"pfb_{rep}_{r}")
                    ob = work.tile([P, 4, H], F32, tag="ob", bufs=2,
                                   name=f"ob_{rep}_{r}")
                    for i in range(4):
                        t = pf_a if i < 2 else pf_b
                        pfs = t[:, i % 2, 0:H + 1]
                        nc.tensor.transpose(
                            pfs, ot[:, i * P:(i + 1) * P],
                            ident[:H + 1, :H + 1],
                        )
                        rs = work.tile([P, 1], F32, tag="rs",
                                       name=f"rs_{rep}_{r}_{i}")
                        nc.vector.reciprocal(rs[:], pfs[:, H:H + 1])
                        nc.vector.tensor_scalar_mul(
                            ob[:, i, :], pfs[:, 0:H], rs[:]
                        )
                    nc.sync.dma_start(
                        out=out_d[r * W:(r + 1) * W, :].rearrange(
                            "(i p) h -> p i h", p=P),
                        in_=ob[:],
                    )

    nc.compile()
    return nc


_NC_CACHE = {}


def _get_nc(reps=1):
    if reps not in _NC_CACHE:
        _NC_CACHE[reps] = build_kernel(reps)
    return _NC_CACHE[reps]


def prep_inputs(input, Wq, Wk, Wv):
    """Host-side prep: cast x to bf16, concat weights. Returns per-core
    input maps keyed by the kernel's DRAM tensor names."""
    import ml_dtypes

    bf16 = ml_dtypes.bfloat16
    x = np.asarray(input)
    B = x.shape[0]
    # x^T layout [p, t, n]: xt[b, p, t, n] = x[b, n, t*128 + p]
    xt = np.ascontiguousarray(
        x.reshape(B, N, DT, P).transpose(0, 3, 2, 1).astype(bf16))
    Wq = np.asarray(Wq, dtype=np.float32)
    Wk = np.asarray(Wk, dtype=np.float32)
    Wv = np.asarray(Wv, dtype=np.float32)
    wqk = np.concatenate([Wq, Wk], axis=1)
    wvq = np.concatenate([Wv, Wq], axis=1)
    # [p, w, t, h]: w2[p, w, t, h] = W[w][t*128 + p, h]
    w2 = np.ascontiguousarray(
        np.stack([wqk, wvq], axis=0).reshape(2, DT, P, 2 * H)
        .transpose(2, 0, 1, 3).astype(bf16))
    return [{"x": xt[b], "W2": w2} for b in range(B)]


def kernel(input, Wq, Wk, Wv, **_unused):
    input = np.asarray(input)
    B = input.shape[0]
    assert B == 8 and input.shape[1] == N and input.shape[2] == D

    nc = _get_nc()
    in_maps = prep_inputs(input, Wq, Wk, Wv)
    res = run_bass_kernel_spmd(nc, in_maps, core_ids=list(range(B)))
    return np.stack([res.results[b]["out"] for b in range(B)], axis=0)


# revision 18
# speedup vs baseline: 1.0923x; 1.0923x over previous
"""Bass/Tile kernel for a single causal attention head on 8 trn2 NeuronCores.

Problem: input [8, 2048, 768], Wq/Wk/Wv [768, 64] ->
  O = softmax(causal(Q K^T)/sqrt(64)) V, per batch.  [8, 2048, 64]

Sharding: data-parallel over batch; core b handles batch b. Weights replicated.

v2 dataflow (all-bf16 matmul path, range-pipelined):
  - Host casts x to bf16 and pre-concatenates weights [Wq|Wk], [Wv|Wq] in
    bf16: two weight DMAs, no on-device weight shuffling.
  - x^T arrives via 24 xbar DMA transposes ([512,128] -> [128,512] chunks,
    one per (query-range, d-block)), issued upfront; projections for range
    g start as soon as its 6 chunks land.
  - Projections accumulate [Wq|Wk]^T xT and [Wv|Qq]^T xT d-major into one
    2-bank PSUM tile per range: rows 0-63 Q^T/V^T, 64-127 K^T/Q^T(dup).
  - Attention per query range r (width 512): S^T pair matmuls (full
    k-tiles) -> one exp (ACT, 1/8 scale fused) per pair; diagonal k-tiles
    packed 3-into-[128,1024] + one [128,256]; invalid (q < k) halves of
    diagonal blocks zeroed by GPSIMD affine_select post-exp.
  - proj(r+1) runs on PE between S(r) and O(r), hiding the exp latency;
    V-transposes for r+1 run after O(r), hiding their DVE-copy dependency.
  - O^T accumulates V_aug_j^T @ expS^T in PSUM; V_aug's ones column makes
    row 64 the softmax denominator.  O^T is PE-transposed back, scaled by
    1/rowsum, and stored with one batched DMA per range.

PSUM budget (8 banks): mm (proj accum / S pairs / S diag) 3x2 + s1
(diag-256 / V-transp / out-transp) 1 + po (O^T accum) 1.
"""

import numpy as np

import concourse.tile as tile
from concourse import bacc, mybir
from concourse.bass_utils import run_bass_kernel_spmd
from concourse.masks import make_identity

P = 128
N = 2048
D = 768
H = 64
NT = N // P   # 16 n-tiles
DT = D // P   # 6 d-tiles
W = 512       # q-range width
QR = N // W   # 4 q-ranges
F32 = mybir.dt.float32
BF16 = mybir.dt.bfloat16
MMDT = BF16
BF16_X = True

# Three of the four diagonal tiles (widths 512, 384, 128) pack into one
# [128, 1024] psum; the 256-wide one (jj=2) gets its own narrow tile.
DIAG_PACK = {0: (0, 512), 1: (512, 384), 3: (896, 128)}  # jj -> (off, width)
DIAG_TOT = 1024


def build_kernel(reps=1):
    nc = bacc.Bacc(name="attn_head")
    x_d = nc.dram_tensor("x", [P, DT, N], BF16, kind="ExternalInput")
    w2_d = nc.dram_tensor("W2", [P, 2, DT, 2 * H], BF16,
                          kind="ExternalInput")
    out_d = nc.dram_tensor("out", [N, H], F32, kind="ExternalOutput")

    Exp = mybir.ActivationFunctionType.Exp

    with tile.TileContext(nc) as tc:
        with (
            tc.tile_pool(name="persist", bufs=1) as persist,
            tc.tile_pool(name="xpool", bufs=1) as xpool,
            tc.tile_pool(name="work", bufs=3) as work,
            tc.tile_pool(name="psum", bufs=1, space="PSUM") as psum,
        ):
            # warm the ACT exp table while DMAs run
            dummy = persist.tile([P, 1], F32)
            nc.vector.memset(dummy[:], 0.0)
            nc.scalar.activation(dummy[:], dummy[:], Exp)

            ident = persist.tile([P, P], F32)
            make_identity(nc, ident[:])
            ident_r = persist.tile([P, P], MMDT)
            nc.vector.tensor_copy(out=ident_r[:], in_=ident[:])

            ones_col = persist.tile([P, 1], F32)
            nc.vector.memset(ones_col[:], 1.0)

            xT = xpool.tile([P, DT, N], BF16)      # x^T: [d%128, d//128, n]
            QK_sb = persist.tile([P, N], MMDT)       # rows 0-63 Q^T, 64- K^T
            VQ_sb = persist.tile([P, N], MMDT)       # rows 0-63 V^T, 64- Q^T
            Vb = persist.tile([P, NT, H + 1], MMDT)  # V tiles + ones col
            nc.vector.tensor_copy(
                out=Vb[:, :, H],
                in_=ones_col[:, 0].to_broadcast((P, NT)),
            )

            # weights as [128, 2, 6, 128]: partition = d%128, tile = d//128,
            # pre-concatenated on host ([Wq|Wk], [Wv|Wq]) so one matmul
            # emits two 64-row outputs; one DMA loads both.
            w_all = persist.tile([P, 2, DT, 2 * H], BF16)
            w_qk = w_all[:, 0]
            w_vq = w_all[:, 1]

            for rep in range(reps):
                # x arrives host-transposed in the xT layout [p, t, n]:
                # plain full-bandwidth DMAs, chunked per query range so
                # projections start as soon as their chunk lands.  Weight
                # halves bracket the first chunk so the QK stream can start
                # right after xt0 and the VQ stream right after.
                def xt_dma(g):
                    nc.sync.dma_start(
                        out=xT[:, :, g * W:(g + 1) * W],
                        in_=x_d[:, :, g * W:(g + 1) * W],
                    )
                if rep == 0:
                    nc.sync.dma_start(out=w_all[:, 0], in_=w2_d[:, 0])
                xt_dma(0)
                if rep == 0:
                    nc.sync.dma_start(out=w_all[:, 1], in_=w2_d[:, 1])
                for g in range(1, QR):
                    xt_dma(g)

                KTd = QK_sb[H:P, :]   # K^T on partitions 64-127
                QTd = VQ_sb[H:P, :]   # Q^T duplicate on partitions 64-127

                def proj_mm(r):
                    ns = slice(r * W, (r + 1) * W)
                    prj = psum.tile([P, 2, W], F32, tag="mm", bufs=3,
                                    name=f"prj_{rep}_{r}")
                    for si, w_t in ((0, w_qk), (1, w_vq)):
                        for d_i in range(DT):
                            kw = dict(start=(d_i == 0), stop=(d_i == DT - 1))
                            nc.tensor.matmul(prj[:, si], w_t[:, d_i],
                                             xT[:, d_i, ns], **kw)
                    nc.vector.tensor_copy(out=QK_sb[:, ns], in_=prj[:, 0])
                    nc.vector.tensor_copy(out=VQ_sb[:, ns], in_=prj[:, 1])

                def vtrans(r):
                    # V^T tiles -> V_aug [128, 65] per k-tile (batched copy)
                    pvt = psum.tile([P, 4, H], MMDT, tag="s1", bufs=1,
                                    name=f"pvt_{rep}_{r}")
                    for i in range(4):
                        nc.tensor.transpose(
                            pvt[:, i, :],
                            VQ_sb[0:H, r * W + i * P:r * W + (i + 1) * P],
                            ident_r[:H, :H],
                        )
                    nc.vector.tensor_copy(out=Vb[:, 4 * r:4 * r + 4, 0:H],
                                          in_=pvt[:])

                proj_mm(0)
                proj_mm(1)
                vtrans(0)

                for r in range(QR):
                    qs = slice(r * W, (r + 1) * W)
                    last_r = (r == QR - 1)
                    pair_es = []
                    diag = {}

                    def s_pairs(r=r, qs=qs, pair_es=pair_es):
                        for jp in range(2 * r):
                            ps2 = psum.tile([P, 2, W], F32, tag="mm", bufs=3,
                                            name=f"ps2_{rep}_{r}_{jp}")
                            es2 = work.tile([P, 2, W], MMDT, tag="es2",
                                            bufs=6, name=f"es2_{rep}_{r}_{jp}")
                            for u in range(2):
                                j = 2 * jp + u
                                nc.tensor.matmul(
                                    ps2[:, u, :], KTd[:, j * P:(j + 1) * P],
                                    QTd[:, qs], start=True, stop=True,
                                )
                            nc.scalar.activation(es2[:], ps2[:], Exp,
                                                 scale=0.125)
                            pair_es.append(es2)

                    def s_diag(r=r, diag=diag):
                        # diagonal k-tiles: 3 packed in [128, 1024] + one
                        # [128, 256]; invalid (q < k) halves of the diagonal
                        # 128x128 blocks zeroed post-exp on GPSIMD.
                        psd = psum.tile([P, DIAG_TOT], F32, tag="mm", bufs=3,
                                        name=f"psd_{rep}_{r}")
                        esd = work.tile([P, DIAG_TOT], MMDT, tag="esd",
                                        bufs=4, name=f"esd_{rep}_{r}")
                        ps1 = psum.tile([P, 256], F32, tag="s1", bufs=1,
                                        name=f"ps1_{rep}_{r}")
                        es1 = work.tile([P, 256], MMDT, tag="es1", bufs=4,
                                        name=f"es1_{rep}_{r}")
                        for jj in range(4):
                            j = 4 * r + jj
                            if jj == 2:
                                sv = ps1[:, :]
                            else:
                                poff, wd = DIAG_PACK[jj]
                                sv = psd[:, poff:poff + wd]
                            nc.tensor.matmul(
                                sv,
                                KTd[:, j * P:(j + 1) * P],
                                QTd[:, r * W + jj * P:(r + 1) * W],
                                start=True, stop=True,
                            )
                        nc.scalar.activation(esd[:, 0:W], psd[:, 0:W], Exp,
                                             scale=0.125)
                        nc.scalar.activation(esd[:, W:], psd[:, W:], Exp,
                                             scale=0.125)
                        nc.scalar.activation(es1[:], ps1[:], Exp, scale=0.125)
                        for jj in range(4):
                            ev = es1[:, 0:P] if jj == 2 else (
                                esd[:, DIAG_PACK[jj][0]:
                                     DIAG_PACK[jj][0] + P])
                            nc.gpsimd.affine_select(
                                out=ev, in_=ev,
                                compare_op=mybir.AluOpType.is_ge,
                                fill=0.0, base=0,
                                pattern=[[1, P]], channel_multiplier=-1,
                            )
                        diag["esd"] = esd
                        diag["es1"] = es1

                    def o_pairs(po, first, last, r=r, pair_es=pair_es):
                        n = 2 * r
                        for jp in range(n):
                            for u in range(2):
                                j = 2 * jp + u
                                nc.tensor.matmul(
                                    po[:], Vb[:, j, :], pair_es[jp][:, u, :],
                                    start=(first and j == 0),
                                    stop=(last and jp == n - 1 and u == 1),
                                    skip_group_check=True,
                                )

                    def o_diag(po, first, last, r=r, diag=diag):
                        for jj in range(4):
                            j = 4 * r + jj
                            if jj == 2:
                                rhs = diag["es1"][:, :]
                            else:
                                poff, wd = DIAG_PACK[jj]
                                rhs = diag["esd"][:, poff:poff + wd]
                            nc.tensor.matmul(
                                po[:, jj * P:],
                                Vb[:, j, :],
                                rhs,
                                start=(first and jj == 0),
                                stop=(last and jj == 3),
                                skip_group_check=True,
                            )

                    # ---- S matmuls + exp; diag first on the last range so
                    # the exp->select->O chain is off the tail. proj(r+2)
                    # fills the exp latency on PE.
                    if last_r:
                        s_diag()
                        s_pairs()
                    else:
                        s_pairs()
                        s_diag()
                    if r + 2 < QR:
                        proj_mm(r + 2)

                    # ---- O^T accumulation ------------------------------
                    po = psum.tile([H + 1, W], F32, tag="po", bufs=1,
                                   name=f"po_{rep}_{r}")
                    if last_r:
                        o_diag(po, first=True, last=False)
                        o_pairs(po, first=False, last=True)
                    else:
                        o_pairs(po, first=(r > 0), last=False)
                        o_diag(po, first=(r == 0), last=True)

                    # ---- V-transposes for r+1 (dep ready by now) -------
                    if r + 1 < QR:
                        vtrans(r + 1)

                    # ---- normalize + batched output --------------------
                    # O^T transposes go into two 2-bank psum tiles so the
                    # four groups land in four distinct banks and don't
                    # serialize behind each other's readers.
                    ot = work.tile([H + 1, W], F32, tag="ot", bufs=2,
                                   name=f"ot_{rep}_{r}")
                    nc.gpsimd.tensor_copy(out=ot[:], in_=po[:])
                    pf_a = psum.tile([P, 2, H + 1], F32, tag="s1", bufs=1,
                                     name=f"pfa_{rep}_{r}")
                    pf_b = psum.tile([P, 2, H + 1], F32, tag="s1", bufs=1,
                                     name=f"pfb_{rep}_{r}")
                    ob = work.tile([P, 4, H], F32, tag="ob", bufs=2,
                                   name=f"ob_{rep}_{r}")
                    for i in range(4):
                        t = pf_a if i < 2 else pf_b
                        pfs = t[:, i % 2, :]
                        nc.tensor.transpose(
                            pfs, ot[:, i * P:(i + 1) * P],
                            ident[:H + 1, :H + 1],
                        )
                        rs = work.tile([P, 1], F32, tag="rs",
                                       name=f"rs_{rep}_{r}_{i}")
                        nc.vector.reciprocal(rs[:], pfs[:, H:H + 1])
                        nc.vector.tensor_scalar_mul(
                            ob[:, i, :], pfs[:, 0:H], rs[:]
                        )
                    nc.sync.dma_start(
                        out=out_d[r * W:(r + 1) * W, :].rearrange(
                            "(i p) h -> p i h", p=P),
                        in_=ob[:],
                    )

    nc.compile()
    return nc


_NC_CACHE = {}


def _get_nc(reps=1):
    if reps not in _NC_CACHE:
        _NC_CACHE[reps] = build_kernel(reps)
    return _NC_CACHE[reps]


def prep_inputs(input, Wq, Wk, Wv):
    """Host-side prep: cast x to bf16, concat weights. Returns per-core
    input maps keyed by the kernel's DRAM tensor names."""
    import ml_dtypes

    bf16 = ml_dtypes.bfloat16
    x = np.asarray(input)
    B = x.shape[0]
    # x^T layout [p, t, n]: xt[b, p, t, n] = x[b, n, t*128 + p]
    xt = np.ascontiguousarray(
        x.reshape(B, N, DT, P).transpose(0, 3, 2, 1).astype(bf16))
    Wq = np.asarray(Wq, dtype=np.float32)
    Wk = np.asarray(Wk, dtype=np.float32)
    Wv = np.asarray(Wv, dtype=np.float32)
    wqk = np.concatenate([Wq, Wk], axis=1)
    wvq = np.concatenate([Wv, Wq], axis=1)
    # [p, w, t, h]: w2[p, w, t, h] = W[w][t*128 + p, h]
    w2 = np.ascontiguousarray(
        np.stack([wqk, wvq], axis=0).reshape(2, DT, P, 2 * H)
        .transpose(2, 0, 1, 3).astype(bf16))
    return [{"x": xt[b], "W2": w2} for b in range(B)]


def kernel(input, Wq, Wk, Wv, **_unused):
    input = np.asarray(input)
    B = input.shape[0]
    assert B == 8 and input.shape[1] == N and input.shape[2] == D

    nc = _get_nc()
    in_maps = prep_inputs(input, Wq, Wk, Wv)
    res = run_bass_kernel_spmd(nc, in_maps, core_ids=list(range(B)))
    return np.stack([res.results[b]["out"] for b in range(B)], axis=0)


# revision 19
# speedup vs baseline: 1.5560x; 1.4246x over previous
"""Bass/Tile kernel for a single causal attention head on 8 trn2 NeuronCores.

Problem: input [8, 2048, 768], Wq/Wk/Wv [768, 64] ->
  O = softmax(causal(Q K^T)/sqrt(64)) V, per batch.  [8, 2048, 64]

Sharding: data-parallel over batch; core b handles batch b. Weights replicated.

v2 dataflow (all-bf16 matmul path, range-pipelined):
  - Host casts x to bf16 and pre-concatenates weights [Wq|Wk], [Wv|Wq] in
    bf16: two weight DMAs, no on-device weight shuffling.
  - x^T arrives via 24 xbar DMA transposes ([512,128] -> [128,512] chunks,
    one per (query-range, d-block)), issued upfront; projections for range
    g start as soon as its 6 chunks land.
  - Projections accumulate [Wq|Wk]^T xT and [Wv|Qq]^T xT d-major into one
    2-bank PSUM tile per range: rows 0-63 Q^T/V^T, 64-127 K^T/Q^T(dup).
  - Attention per query range r (width 512): S^T pair matmuls (full
    k-tiles) -> one exp (ACT, 1/8 scale fused) per pair; diagonal k-tiles
    packed 3-into-[128,1024] + one [128,256]; invalid (q < k) halves of
    diagonal blocks zeroed by GPSIMD affine_select post-exp.
  - proj(r+1) runs on PE between S(r) and O(r), hiding the exp latency;
    V-transposes for r+1 run after O(r), hiding their DVE-copy dependency.
  - O^T accumulates V_aug_j^T @ expS^T in PSUM; V_aug's ones column makes
    row 64 the softmax denominator.  O^T is PE-transposed back, scaled by
    1/rowsum, and stored with one batched DMA per range.

PSUM budget (8 banks): mm (proj accum / S pairs / S diag) 3x2 + s1
(diag-256 / V-transp / out-transp) 1 + po (O^T accum) 1.
"""

import numpy as np

import concourse.tile as tile
from concourse import bacc, mybir
from concourse.bass_utils import run_bass_kernel_spmd
from concourse.masks import make_identity

P = 128
N = 2048
D = 768
H = 64
NT = N // P   # 16 n-tiles
DT = D // P   # 6 d-tiles
W = 512       # q-range width
QR = N // W   # 4 q-ranges
F32 = mybir.dt.float32
BF16 = mybir.dt.bfloat16
MMDT = BF16
BF16_X = True

# Three of the four diagonal tiles (widths 512, 384, 128) pack into one
# [128, 1024] psum; the 256-wide one (jj=2) gets its own narrow tile.
DIAG_PACK = {0: (0, 512), 1: (512, 384), 3: (896, 128)}  # jj -> (off, width)
DIAG_TOT = 1024


def build_kernel(reps=1):
    nc = bacc.Bacc(name="attn_head")
    x_d = nc.dram_tensor("x", [P, DT, N], BF16, kind="ExternalInput")
    w2_d = nc.dram_tensor("W2", [P, 2, DT, 2 * H], BF16,
                          kind="ExternalInput")
    out_d = nc.dram_tensor("out", [N, H], F32, kind="ExternalOutput")

    Exp = mybir.ActivationFunctionType.Exp

    with tile.TileContext(nc) as tc:
        with (
            tc.tile_pool(name="persist", bufs=1) as persist,
            tc.tile_pool(name="xpool", bufs=1) as xpool,
            tc.tile_pool(name="work", bufs=3) as work,
            tc.tile_pool(name="psum", bufs=1, space="PSUM") as psum,
        ):
            # warm the ACT exp table while DMAs run
            dummy = persist.tile([P, 1], F32)
            nc.vector.memset(dummy[:], 0.0)
            nc.scalar.activation(dummy[:], dummy[:], Exp)

            ident = persist.tile([P, P], F32)
            make_identity(nc, ident[:])
            ident_r = persist.tile([P, P], MMDT)
            nc.vector.tensor_copy(out=ident_r[:], in_=ident[:])

            ones_col = persist.tile([P, 1], F32)
            nc.vector.memset(ones_col[:], 1.0)

            xT = xpool.tile([P, DT, N], BF16)      # x^T: [d%128, d//128, n]
            QK_sb = persist.tile([P, N], MMDT)       # rows 0-63 Q^T, 64- K^T
            VQ_sb = persist.tile([P, N], MMDT)       # rows 0-63 V^T, 64- Q^T
            Vb = persist.tile([P, NT, H + 1], MMDT)  # V tiles + ones col
            nc.vector.tensor_copy(
                out=Vb[:, :, H],
                in_=ones_col[:, 0].to_broadcast((P, NT)),
            )

            # weights as [128, 2, 6, 128]: partition = d%128, tile = d//128,
            # pre-concatenated on host ([Wq|Wk], [Wv|Wq]) so one matmul
            # emits two 64-row outputs; one DMA loads both.
            w_all = persist.tile([P, 2, DT, 2 * H], BF16)
            w_qk = w_all[:, 0]
            w_vq = w_all[:, 1]

            for rep in range(reps):
                # x arrives host-transposed in the xT layout [p, t, n]:
                # plain full-bandwidth DMAs, chunked per query range so
                # projections start as soon as their chunk lands.  Weight
                # halves bracket the first chunk so the QK stream can start
                # right after xt0 and the VQ stream right after.
                def xt_dma(g):
                    nc.sync.dma_start(
                        out=xT[:, :, g * W:(g + 1) * W],
                        in_=x_d[:, :, g * W:(g + 1) * W],
                    )
                if rep == 0:
                    nc.sync.dma_start(out=w_all[:, 0], in_=w2_d[:, 0])
                xt_dma(0)
                if rep == 0:
                    nc.sync.dma_start(out=w_all[:, 1], in_=w2_d[:, 1])
                for g in range(1, QR):
                    xt_dma(g)

                KTd = QK_sb[H:P, :]   # K^T on partitions 64-127
                QTd = VQ_sb[H:P, :]   # Q^T duplicate on partitions 64-127

                def proj_mm(r):
                    ns = slice(r * W, (r + 1) * W)
                    prj = psum.tile([P, 2, W], F32, tag="mm", bufs=3,
                                    name=f"prj_{rep}_{r}")
                    for si, w_t in ((0, w_qk), (1, w_vq)):
                        for d_i in range(DT):
                            kw = dict(start=(d_i == 0), stop=(d_i == DT - 1))
                            nc.tensor.matmul(prj[:, si], w_t[:, d_i],
                                             xT[:, d_i, ns], **kw)
                    nc.vector.tensor_copy(out=QK_sb[:, ns], in_=prj[:, 0])
                    nc.vector.tensor_copy(out=VQ_sb[:, ns], in_=prj[:, 1])

                def vtrans(r):
                    # V^T tiles -> V_aug [128, 65] per k-tile (batched copy)
                    pvt = psum.tile([P, 4, H], MMDT, tag="s1", bufs=1,
                                    name=f"pvt_{rep}_{r}")
                    for i in range(4):
                        nc.tensor.transpose(
                            pvt[:, i, :],
                            VQ_sb[0:H, r * W + i * P:r * W + (i + 1) * P],
                            ident_r[:H, :H],
                        )
                    nc.vector.tensor_copy(out=Vb[:, 4 * r:4 * r + 4, 0:H],
                                          in_=pvt[:])

                proj_mm(0)
                proj_mm(1)
                vtrans(0)

                for r in range(QR):
                    qs = slice(r * W, (r + 1) * W)
                    last_r = (r == QR - 1)
                    pair_es = []
                    diag = {}

                    def s_pairs(r=r, qs=qs, pair_es=pair_es):
                        for jp in range(2 * r):
                            ps2 = psum.tile([P, 2, W], F32, tag="mm", bufs=3,
                                            name=f"ps2_{rep}_{r}_{jp}")
                            es2 = work.tile([P, 2, W], MMDT, tag="es2",
                                            bufs=6, name=f"es2_{rep}_{r}_{jp}")
                            for u in range(2):
                                j = 2 * jp + u
                                nc.tensor.matmul(
                                    ps2[:, u, :], KTd[:, j * P:(j + 1) * P],
                                    QTd[:, qs], start=True, stop=True,
                                )
                            nc.scalar.activation(es2[:], ps2[:], Exp,
                                                 scale=0.125)
                            pair_es.append(es2)

                    def s_diag(r=r, diag=diag):
                        # diagonal k-tiles: 3 packed in [128, 1024] + one
                        # [128, 256]; invalid (q < k) halves of the diagonal
                        # 128x128 blocks zeroed post-exp on GPSIMD.
                        psd = psum.tile([P, DIAG_TOT], F32, tag="mm", bufs=3,
                                        name=f"psd_{rep}_{r}")
                        esd = work.tile([P, DIAG_TOT], MMDT, tag="esd",
                                        bufs=4, name=f"esd_{rep}_{r}")
                        ps1 = psum.tile([P, 256], F32, tag="s1", bufs=1,
                                        name=f"ps1_{rep}_{r}")
                        es1 = work.tile([P, 256], MMDT, tag="es1", bufs=4,
                                        name=f"es1_{rep}_{r}")
                        for jj in range(4):
                            j = 4 * r + jj
                            if jj == 2:
                                sv = ps1[:, :]
                            else:
                                poff, wd = DIAG_PACK[jj]
                                sv = psd[:, poff:poff + wd]
                            nc.tensor.matmul(
                                sv,
                                KTd[:, j * P:(j + 1) * P],
                                QTd[:, r * W + jj * P:(r + 1) * W],
                                start=True, stop=True,
                            )
                        nc.scalar.activation(esd[:, 0:W], psd[:, 0:W], Exp,
                                             scale=0.125)
                        nc.scalar.activation(esd[:, W:], psd[:, W:], Exp,
                                             scale=0.125)
                        nc.scalar.activation(es1[:], ps1[:], Exp, scale=0.125)
                        for jj in range(4):
                            ev = es1[:, 0:P] if jj == 2 else (
                                esd[:, DIAG_PACK[jj][0]:
                                     DIAG_PACK[jj][0] + P])
                            nc.gpsimd.affine_select(
                                out=ev, in_=ev,
                                compare_op=mybir.AluOpType.is_ge,
                                fill=0.0, base=0,
                                pattern=[[1, P]], channel_multiplier=-1,
                            )
                        diag["esd"] = esd
                        diag["es1"] = es1

                    def o_pairs(po, first, last, r=r, pair_es=pair_es):
                        n = 2 * r
                        for jp in range(n):
                            for u in range(2):
                                j = 2 * jp + u
                                nc.tensor.matmul(
                                    po[:], Vb[:, j, :], pair_es[jp][:, u, :],
                                    start=(first and j == 0),
                                    stop=(last and jp == n - 1 and u == 1),
                                    skip_group_check=True,
                                )

                    def o_diag(po, first, last, r=r, diag=diag):
                        for jj in range(4):
                            j = 4 * r + jj
                            if jj == 2:
                                rhs = diag["es1"][:, :]
                            else:
                                poff, wd = DIAG_PACK[jj]
                                rhs = diag["esd"][:, poff:poff + wd]
                            nc.tensor.matmul(
                                po[:, jj * P:],
                                Vb[:, j, :],
                                rhs,
                                start=(first and jj == 0),
                                stop=(last and jj == 3),
                                skip_group_check=True,
                            )

                    # ---- S matmuls + exp; diag first on the last range so
                    # the exp->select->O chain is off the tail. proj(r+2)
                    # fills the exp latency on PE.
                    if last_r:
                        s_diag()
                        s_pairs()
                    else:
                        s_pairs()
                        s_diag()
                    if r + 2 < QR:
                        proj_mm(r + 2)

                    # ---- O^T accumulation ------------------------------
                    po = psum.tile([H + 1, W], F32, tag="po", bufs=1,
                                   name=f"po_{rep}_{r}")
                    if last_r:
                        o_diag(po, first=True, last=False)
                        o_pairs(po, first=False, last=True)
                    else:
                        o_pairs(po, first=(r > 0), last=False)
                        o_diag(po, first=(r == 0), last=True)

                    # ---- V-transposes for r+1 (dep ready by now) -------
                    if r + 1 < QR:
                        vtrans(r + 1)

                    # ---- normalize + batched output --------------------
                    # O^T transposes go into two 2-bank psum tiles so the
                    # four groups land in four distinct banks and don't
                    # serialize behind each other's readers.
                    ot = work.tile([H + 1, W], F32, tag="ot", bufs=2,
                                   name=f"ot_{rep}_{r}")
                    nc.vector.tensor_copy(out=ot[:], in_=po[:])
                    pf_a = psum.tile([P, 2, H + 1], F32, tag="s1", bufs=1,
                                     name=f"pfa_{rep}_{r}")
                    pf_b = psum.tile([P, 2, H + 1], F32, tag="s1", bufs=1,
                                     name=f"pfb_{rep}_{r}")
                    ob = work.tile([P, 4, H], F32, tag="ob", bufs=2,
                                   name=f"ob_{rep}_{r}")
                    for i in range(4):
                        t = pf_a if i < 2 else pf_b
                        pfs = t[:, i % 2, :]
                        nc.tensor.transpose(
                            pfs, ot[:, i * P:(i + 1) * P],
                            ident[:H + 1, :H + 1],
                        )
                        rs = work.tile([P, 1], F32, tag="rs",
                                       name=f"rs_{rep}_{r}_{i}")
                        nc.vector.reciprocal(rs[:], pfs[:, H:H + 1])
                        nc.vector.tensor_scalar_mul(
                            ob[:, i, :], pfs[:, 0:H], rs[:]
                        )
                    nc.sync.dma_start(
                        out=out_d[r * W:(r + 1) * W, :].rearrange(
                            "(i p) h -> p i h", p=P),
                        in_=ob[:],
                    )

    nc.compile()
    return nc


_NC_CACHE = {}


def _get_nc(reps=1):
    if reps not in _NC_CACHE:
        _NC_CACHE[reps] = build_kernel(reps)
    return _NC_CACHE[reps]


def prep_inputs(input, Wq, Wk, Wv):
    """Host-side prep: cast x to bf16, concat weights. Returns per-core
    input maps keyed by the kernel's DRAM tensor names."""
    import ml_dtypes

    bf16 = ml_dtypes.bfloat16
    x = np.asarray(input)
    B = x.shape[0]
    # x^T layout [p, t, n]: xt[b, p, t, n] = x[b, n, t*128 + p]
    xt = np.ascontiguousarray(
        x.reshape(B, N, DT, P).transpose(0, 3, 2, 1).astype(bf16))
    Wq = np.asarray(Wq, dtype=np.float32)
    Wk = np.asarray(Wk, dtype=np.float32)
    Wv = np.asarray(Wv, dtype=np.float32)
    wqk = np.concatenate([Wq, Wk], axis=1)
    wvq = np.concatenate([Wv, Wq], axis=1)
    # [p, w, t, h]: w2[p, w, t, h] = W[w][t*128 + p, h]
    w2 = np.ascontiguousarray(
        np.stack([wqk, wvq], axis=0).reshape(2, DT, P, 2 * H)
        .transpose(2, 0, 1, 3).astype(bf16))
    return [{"x": xt[b], "W2": w2} for b in range(B)]


def kernel(input, Wq, Wk, Wv, **_unused):
    input = np.asarray(input)
    B = input.shape[0]
    assert B == 8 and input.shape[1] == N and input.shape[2] == D

    nc = _get_nc()
    in_maps = prep_inputs(input, Wq, Wk, Wv)
    res = run_bass_kernel_spmd(nc, in_maps, core_ids=list(range(B)))
    return np.stack([res.results[b]["out"] for b in range(B)], axis=0)
